# revision 16
# baseline (speedup 1.0000x reference)
"""Causal multi-head attention block (qkv proj + attention + out proj) on 8 TRN2 cores.

Problem: x[4,2048,1024] @ Wqkv[1024,3072] -> 16-head causal attention -> @ Wout.

Sharding: batch(4) x head-group(2) -> 8 cores. Core c handles batch c//2 and
heads (c%2)*8..(c%2)*8+8. Each core computes its 8 heads' attention and a
partial out-projection [2048,1024]; host sums the two head-group partials per
batch and adds bout.

v2 over the ~300us v1 baseline:
  - Q/K projections in fp8e4 DoubleRow (both operands fp8, 2 contraction
    chunks per instr): 2x PE throughput on mm_q/mm_k. Wq,Wk pre-scaled x32
    on host so fp8 operands use the e4m3 range (|psq| <~ 123 < 240).
  - Q,K stored fp8 (x32); mm_s in fp8 (runs at bf16 rate) with ROW TILING:
    head hb=0 lives at partitions 0-63, hb=1 at 64-127; adjacent mm_s
    instructions target disjoint row groups and execute CONCURRENTLY on the
    PE (measured 109ns vs 214ns per pair) -> 2x on mm_s. unit2() interleaves
    both heads of an hp pair to make the pairs adjacent.
  - exp ACT carries scale=1/8192 (undo 32*32 and the 1/sqrt(64)).
  - V path / ctx / out-proj stay bf16 (fp8 there fails the 2e-2 tolerance).
  - Causal slicing everywhere; masks shrink to one [128,128] triangle block.
  - Softmax denominator rides as V's 65th column through the ctx matmul.
  - Projection + out-proj matmuls are deadline-scheduled into the attention
    inner loop via a paced background closure queue.
  - PSUM: 2x[128,2,512] S (one per head of the pair) + 2x[128,512] proj +
    2x[65,512] ctx = 8 banks.
"""
import numpy as np

B, T, C = 4, 2048, 1024
H, HD = 16, 64
NCORES = 8
WSCALE = 32.0
EXP_SCALE = 1.0 / (WSCALE * WSCALE * 8.0)
DVE_EXP_EVERY = 0  # 0=off; else every Nth exp emission runs on DVE


def _fit_exp16_constants(lam, smax=3.65):
    """Degree-2 near-minimax fit q(v)~e^v on v=x*lam/16; exp(x*lam)=q(x)^16.
    Returns (c0, c1, c2) on raw x. Max rel err ~0.8% after ^16."""
    vmax = smax / 16.0
    v = np.linspace(-vmax, vmax, 20001)
    t = np.exp(v)
    w = 1.0 / t
    for _ in range(60):
        A = np.stack([np.ones_like(v), v, v * v], axis=1) * w[:, None]
        c = np.linalg.lstsq(A, t * w, rcond=None)[0]
        relerr = np.abs((c[0] + c[1] * v + c[2] * v * v) / t - 1.0)
        w = w * (1.0 + 2.0 * relerr / max(relerr.max(), 1e-12))
        w /= w.max()
    k = lam / 16.0
    return float(c[0]), float(c[1] * k), float(c[2] * k * k)


def _register_exp16():
    """Custom DVE op out = ((c2 x + c1) x + c0)^16 ~ exp(x*EXP_SCALE).
    8 ALU stages (v3 budget). Appended to concourse.dve_ops at runtime."""
    import concourse.dve_ops as dops
    from concourse.dve_spec import Spec, Src0, C0, C1, C2, lower, sq, _has_src1
    from concourse.dve_uop import DveOpSpec

    name = "EXP_PWR16_ANT"
    for op in dops.OPS:
        if op.name == name:
            return op
    body = sq(sq(sq(sq((C2 * Src0 + C1) * Src0 + C0))))

    def _ref(in0, in1, s0, s1, imm2):
        q = (imm2 * in0 + s1) * in0 + s0
        return q ** 16

    spec = Spec(body=body, reference=_ref)
    opcode = dops._CUSTOM_DVE_ROW_BASE + len(dops.OPS)
    shas = {}
    for ver in ("v3", "v4"):
        try:
            compiled = DveOpSpec(name=name, opcode=opcode,
                                 uops=lower(spec, ver=ver),
                                 rd1_en=_has_src1(spec))
            shas[ver] = compiled.sha(ver)
        except Exception:
            pass
    op = dops.DveOp(name, spec, subdim=False, uops_sha=shas)
    dops.OPS.append(op)
    dops.CUSTOM_DVE_SPECS[name] = spec
    dops._SUB_OPCODE_FOR_NAME[name] = opcode
    return op


def _build_program():
    import concourse.bacc as bacc
    import concourse.tile as tile
    from concourse import mybir

    dtf = mybir.dt.float32
    dtr = mybir.dt.float32r
    dtb = mybir.dt.bfloat16
    dt8 = mybir.dt.float8e4
    EXP = mybir.ActivationFunctionType.Exp
    MULT = mybir.AluOpType.mult
    DR = mybir.MatmulPerfMode.DoubleRow

    exp16 = _register_exp16()
    ec0, ec1, ec2 = _fit_exp16_constants(EXP_SCALE)
    expctr = {'i': 0}

    nc = bacc.Bacc('TRN2', target_bir_lowering=False, debug=False)
    xt8_d = nc.dram_tensor("xt8", [1024, 2048], dt8, kind="ExternalInput").ap()
    xt_d = nc.dram_tensor("xt", [1024, 2048], dtb, kind="ExternalInput").ap()
    wqk_d = nc.dram_tensor("wqk", [1024, 1024], dt8, kind="ExternalInput").ap()
    wv_d = nc.dram_tensor("wv", [1024, 512], dtb, kind="ExternalInput").ap()
    wout_d = nc.dram_tensor("wout", [512, 1024], dtb, kind="ExternalInput").ap()
    mask_d = nc.dram_tensor("mask", [4, 128, 512], dtb, kind="ExternalInput").ap()
    y_d = nc.dram_tensor("y", [2048, 1024], dtf, kind="ExternalOutput").ap()

    with tile.TileContext(nc) as tc:
        with tc.tile_pool(name="ps_s", bufs=2, space="PSUM") as ps_s, \
             tc.tile_pool(name="ps", bufs=2, space="PSUM") as ps, \
             tc.tile_pool(name="ps_ctx", bufs=2, space="PSUM") as ps_ctx, \
             tc.tile_pool(name="const", bufs=1) as const, \
             tc.tile_pool(name="xt_p", bufs=16) as xt_p, \
             tc.tile_pool(name="xt8_p", bufs=8) as xt8_p, \
             tc.tile_pool(name="qt_p", bufs=8) as qt_p, \
             tc.tile_pool(name="exp_p", bufs=8) as exp_p, \
             tc.tile_pool(name="cn_p", bufs=16) as cn_p, \
             tc.tile_pool(name="row_p", bufs=4) as row_p, \
             tc.tile_pool(name="rcp_p", bufs=2) as rcp_p, \
             tc.tile_pool(name="y_p", bufs=2) as y_p:

            # ---- constants / weights ----
            # wqk_sb[p, kp, i, oc, c]: fp8 DR layout; rows 256*kp+128*i+p of
            # Wqk, cols oc*128+c. oc 0-3 Q, 4-7 K.
            wqk_sb = const.tile([128, 4, 2, 8, 128], dt8)
            wv_sb = const.tile([128, 8, 512], dtb)       # [p, kc, n]
            wout_sb = const.tile([128, 4, 2, 512], dtb)  # [p, hp, oc, c]
            masks = const.tile([128, 4, 512], dtb)
            ones_f32 = const.tile([1, 128], dtf)
            ones_t = const.tile([1, 128], dtr)
            kt_store = const.tile([128, 4, 4, 512], dt8)  # [p, hp, tt, t]
            v_all = const.tile([128, 16, 8, 65], dtb)     # [p, kt, h, d|1]

            nc.vector.memset(v_all[:, :, :, 64:65], 1.0)
            nc.vector.memset(ones_f32[:], 1.0)
            nc.vector.tensor_copy(ones_t[:], ones_f32[:])

            qts = {}   # tt -> [4 qt tiles (fp8)]
            xts = {}   # tt -> [8 bf16 xt tiles]
            x8s = {}   # tt -> [4 fp8 xt DR tiles]
            cns = {}   # qt -> [4 cn tiles]

            def emit_xt_dma(tt):
                ts = []
                for kc in range(8):
                    t_ = xt_p.tile([128, 512], dtb, tag="xt")
                    nc.sync.dma_start(
                        t_[:], xt_d[kc * 128:(kc + 1) * 128,
                                    tt * 512:(tt + 1) * 512])
                    ts.append(t_)
                xts[tt] = ts
                t8s = []
                for kp in range(4):
                    t_ = xt8_p.tile([128, 2, 512], dt8, tag="xt8")
                    nc.scalar.dma_start(
                        t_[:], xt8_d[kp * 256:(kp + 1) * 256,
                                     tt * 512:(tt + 1) * 512]
                        .rearrange("(i p) t -> p i t", p=128))
                    t8s.append(t_)
                x8s[tt] = t8s

            # DMA order: Q-half wqk chunks (sync) + xt0 feed the first Q
            # matmuls ASAP; K-half wqk rides the gpsimd queue.
            xts[0] = []
            x8s[0] = []
            for kp in range(4):
                nc.sync.dma_start(
                    wqk_sb[:, kp, :, 0:4, :],
                    wqk_d[kp * 256:(kp + 1) * 256, 0:512]
                    .rearrange("(i p) (oc c) -> p i oc c", p=128, c=128))
                t_ = xt8_p.tile([128, 2, 512], dt8, tag="xt8", name="xt80")
                nc.scalar.dma_start(
                    t_[:], xt8_d[kp * 256:(kp + 1) * 256, 0:512]
                    .rearrange("(i p) t -> p i t", p=128))
                x8s[0].append(t_)
                nc.gpsimd.dma_start(
                    wqk_sb[:, kp, :, 4:8, :],
                    wqk_d[kp * 256:(kp + 1) * 256, 512:1024]
                    .rearrange("(i p) (oc c) -> p i oc c", p=128, c=128))
            for kc in range(8):
                t_ = xt_p.tile([128, 512], dtb, tag="xt", name="xt0")
                nc.scalar.dma_start(
                    t_[:], xt_d[kc * 128:(kc + 1) * 128, 0:512])
                xts[0].append(t_)
            nc.sync.dma_start(wv_sb[:],
                              wv_d.rearrange("(kc p) n -> p kc n", p=128))
            nc.sync.dma_start(masks[:], mask_d.rearrange("n p f -> p n f"))
            emit_xt_dma(1)
            nc.sync.dma_start(wout_sb[:],
                              wout_d.rearrange("(hp p) (oc c) -> p hp oc c",
                                               p=128, c=512))

            def proj_closures(tt):
                cl = []
                qts[tt] = [None] * 4
                state = {}

                def q_mm(j, kp):
                    if kp == 0:
                        state[('q', j)] = ps.tile([128, 512], dtf, tag="ps", name="psq")
                    psq = state[('q', j)]
                    nc.tensor.matmul(psq[:], wqk_sb[:, kp, :, j, :],
                                     x8s[tt][kp][:], start=(kp == 0),
                                     stop=(kp == 3),
                                     perf_mode=DR).annotate('mm_q')
                    if kp == 3:
                        qt_t = qt_p.tile([128, 512], dt8, tag="qt")
                        nc.vector.tensor_copy(qt_t[:], psq[:])
                        qts[tt][j] = qt_t

                def k_mm(j, kp):
                    if kp == 0:
                        state[('k', j)] = ps.tile([128, 512], dtf, tag="ps", name="psk")
                    psk = state[('k', j)]
                    nc.tensor.matmul(psk[:], wqk_sb[:, kp, :, 4 + j, :],
                                     x8s[tt][kp][:], start=(kp == 0),
                                     stop=(kp == 3),
                                     perf_mode=DR).annotate('mm_k')
                    if kp == 3:
                        nc.vector.tensor_copy(kt_store[:, j, tt, :], psk[:])

                def v_mm(sub, kc):
                    if kc == 0:
                        state[('v', sub)] = ps.tile([128, 512], dtf, tag="ps", name="psv")
                    psv = state[('v', sub)]
                    nc.tensor.matmul(psv[:],
                                     xts[tt][kc][:, sub * 128:(sub + 1) * 128],
                                     wv_sb[:, kc, :], start=(kc == 0),
                                     stop=(kc == 7)).annotate('mm_v')
                    if kc == 7:
                        vt = tt * 4 + sub
                        nc.vector.tensor_copy(
                            v_all[:, vt, :, 0:64],
                            psv[:].rearrange("p (h d) -> p h d", h=8))

                for j in range(4):
                    for kp in range(4):
                        cl.append(lambda j=j, kp=kp: q_mm(j, kp))
                for j in range(4):
                    for kp in range(4):
                        cl.append(lambda j=j, kp=kp: k_mm(j, kp))
                for sub in range(4):
                    for kc in range(8):
                        cl.append(lambda s=sub, kc=kc: v_mm(s, kc))
                return cl

            def y_closures(qt):
                cl = []
                state = {}

                def y_mm(mi, oc, hp):
                    if hp == 0:
                        state[(mi, oc)] = ps.tile([128, 512], dtf, tag="ps", name="psy")
                    psy = state[(mi, oc)]
                    nc.tensor.matmul(psy[:],
                                     cns[qt][hp][:, mi * 128:(mi + 1) * 128],
                                     wout_sb[:, hp, oc, :],
                                     start=(hp == 0), stop=(hp == 3)).annotate('mm_y')
                    if hp == 3:
                        y_sb = y_p.tile([128, 512], dtf, tag="y")
                        nc.vector.tensor_copy(y_sb[:], psy[:])
                        nc.sync.dma_start(
                            y_d[qt * 512 + mi * 128: qt * 512 + (mi + 1) * 128,
                                oc * 512:(oc + 1) * 512],
                            y_sb[:])

                for mi in range(4):
                    for oc in range(2):
                        for hp in range(4):
                            cl.append(lambda m=mi, o=oc, h=hp: y_mm(m, o, h))
                return cl

            def unit2(qt, hp, cn_t, pull):
                """Both hb heads of pair hp, interleaved so mm_s for hb=0
                (rows 0-63) and hb=1 (rows 64-127) are pc-adjacent and run
                concurrently on disjoint PE row groups."""
                n_kt = 4 * qt + 4
                np_ = n_kt // 2
                ctx = [ps_ctx.tile([65, 512], dtf, tag="ctx", name=f"ctx{hb}")
                       for hb in (0, 1)]
                pending = []

                def off(kt):
                    di = kt - 4 * qt
                    return 128 * di if di > 0 else 0

                def emit_ctx(j, exs):
                    for hb in (0, 1):
                        for t in (0, 1):
                            kt = 2 * j + t
                            o = off(kt)
                            nc.tensor.matmul(ctx[hb][:, o:512],
                                             v_all[:, kt, 2 * hp + hb, :],
                                             exs[hb][:, t, o:512],
                                             start=(kt == 0),
                                             stop=(kt == n_kt - 1)).annotate('mm_ctx')

                for j in range(np_):
                    sps = [ps_s.tile([128, 2, 512], dtf, tag="s", name=f"sp{hb}")
                           for hb in (0, 1)]
                    for t in (0, 1):
                        kt = 2 * j + t
                        ktt, kj = kt // 4, kt % 4
                        o = off(kt)
                        for hb in (0, 1):
                            pb = hb * 64
                            nc.tensor.matmul(
                                sps[hb][:, t, o:512],
                                kt_store[pb:pb + 64, hp, ktt,
                                         kj * 128:(kj + 1) * 128],
                                qts[qt][hp][pb:pb + 64, o:512],
                                start=True, stop=True).annotate('mm_s')
                    pull(4)
                    exs = []
                    for hb in (0, 1):
                        ex = exp_p.tile([128, 2, 512], dtb, tag="exp")
                        expctr['i'] += 1
                        on_dve = DVE_EXP_EVERY and (expctr['i'] % DVE_EXP_EVERY) == 0

                        def _exp(dst, src):
                            if on_dve:
                                nc.vector._custom_dve(exp16, out=dst, in0=src,
                                                      s0=ec0, s1=ec1, imm2=ec2)
                            else:
                                nc.scalar.activation(dst, src, EXP,
                                                     scale=EXP_SCALE)
                        if off(2 * j) >= 256:
                            for t in (0, 1):
                                o = off(2 * j + t)
                                _exp(ex[:, t, o:512], sps[hb][:, t, o:512])
                        else:
                            _exp(ex[:], sps[hb][:])
                        for t in (0, 1):
                            kt = 2 * j + t
                            di = kt - 4 * qt
                            if 0 <= di < 4:
                                o, w = 128 * di, 128 * (di + 1)
                                nc.vector.tensor_tensor(ex[:, t, o:w],
                                                        ex[:, t, o:w],
                                                        masks[:, di, o:w], MULT)
                        exs.append(ex)
                    pending.append((j, exs))
                    if len(pending) > 2:
                        emit_ctx(*pending.pop(0))
                        pull(2)
                while pending:
                    emit_ctx(*pending.pop(0))
                    pull(2)
                # normalize both heads' halves of cn
                for hb in (0, 1):
                    pb = hb * 64
                    if qt == 3 and hp >= 2:
                        # final units feed the epilogue now: broadcast on PE
                        row = row_p.tile([1, 512], dtr, tag="row")
                        nc.vector.tensor_copy(row[:], ctx[hb][64:65, :])
                        bc = ps.tile([128, 512], dtf, tag="ps", name="bc")
                        nc.tensor.matmul(bc[:], ones_t[:], row[:],
                                         start=True, stop=True).annotate('mm_bc')
                        rcp = rcp_p.tile([64, 512], dtf, tag="rcp", name="rcp")
                        nc.vector.reciprocal_approx_fast(rcp[:], bc[0:64, :])
                        nc.vector.tensor_tensor(cn_t[pb:pb + 64, :],
                                                ctx[hb][0:64, :], rcp[:], MULT)
                        continue
                    row = row_p.tile([1, 512], dtf, tag="row")
                    nc.vector.tensor_copy(row[:], ctx[hb][64:65, :])
                    rrow = row_p.tile([1, 512], dtf, tag="row", name="rrow")
                    nc.vector.reciprocal_approx_fast(rrow[:], row[:])
                    pull(1)
                    rcp = rcp_p.tile([64, 512], dtf, tag="rcp")
                    nc.gpsimd.partition_broadcast(rcp[:], rrow[:])
                    nc.vector.tensor_tensor(cn_t[pb:pb + 64, :],
                                            ctx[hb][0:64, :], rcp[:], MULT)

            def make_puller(items, total_slots):
                st = {'i': 0, 'slot': 0}
                n = len(items)

                def pull(k):
                    st['slot'] += k
                    if total_slots > 0:
                        target = min(n, (n * st['slot'] + total_slots - 1)
                                     // total_slots)
                    else:
                        target = n
                    # burst to a multiple of 8: longer same-type matmul runs
                    # keep the PE weight double-buffer streaming
                    if st['i'] < target:
                        target = min(n, ((target + 15) // 16) * 16)
                    while st['i'] < target:
                        items[st['i']]()
                        st['i'] += 1

                def drain():
                    while st['i'] < n:
                        items[st['i']]()
                        st['i'] += 1
                return pull, drain

            # ---- prologue: tt=0 projections ----
            warm = row_p.tile([1, 1], dtf, tag="row", name="warm")
            nc.scalar.activation(warm[:], v_all[0:1, 0, 0, 64:65], EXP)
            for c in proj_closures(0):
                c()

            # ---- main loop ----
            for qt in range(4):
                Bq = []
                if qt < 2:
                    emit_xt_dma(qt + 2)
                if qt < 3:
                    Bq += proj_closures(qt + 1)
                if qt == 3:
                    Bq += (y_closures(0) + y_closures(1) + y_closures(2)
                           + y_closures(3))
                np_ = (4 * qt + 4) // 2
                total_slots = 4 * (8 * np_ + 2)
                pull, drain = make_puller(Bq, total_slots)
                # pre-register all four cn tiles so late-paced y(3)
                # closures can reference cns[3][hp] before unit2(hp) runs;
                # the tile framework orders mm_y after the cn writes.
                cns[qt] = [cn_p.tile([128, 512], dtb, tag="cn", name=f"cn{hp}")
                           for hp in range(4)]
                for hp in range(4):
                    unit2(qt, hp, cns[qt][hp], pull)
                drain()
    nc.compile()
    return nc


def _host_shards(x, Wqkv, bqkv, Wout):
    import ml_dtypes
    mask = np.zeros((4, 128, 512), np.float32)
    qq = np.arange(512)[None, :]
    kk = np.arange(128)[:, None]
    for di in range(4):
        mask[di] = (kk + di * 128 <= qq)
    mask = mask.astype(ml_dtypes.bfloat16)

    assert not np.any(bqkv), "kernel assumes zero qkv bias"

    in_maps = []
    for c in range(NCORES):
        b, hg = c // 2, c % 2
        s = hg * 512
        xt_f = np.ascontiguousarray(x[b].T)
        xt8 = xt_f.astype(ml_dtypes.float8_e4m3)
        xt = xt_f.astype(ml_dtypes.bfloat16)
        wqk = np.ascontiguousarray(
            np.concatenate([Wqkv[:, s:s + 512] * WSCALE,
                            Wqkv[:, 1024 + s:1024 + s + 512] * WSCALE],
                           axis=1)).astype(ml_dtypes.float8_e4m3)
        wv = np.ascontiguousarray(Wqkv[:, 2048 + s:2048 + s + 512]).astype(ml_dtypes.bfloat16)
        wout = np.ascontiguousarray(Wout[s:s + 512, :]).astype(ml_dtypes.bfloat16)
        in_maps.append({"xt8": xt8, "xt": xt, "wqk": wqk, "wv": wv,
                        "wout": wout, "mask": mask})
    return in_maps


_CACHED = {}


def kernel(x, Wqkv, bqkv, Wout, bout):
    from concourse.bass_utils import run_bass_kernel_spmd

    x = np.asarray(x, dtype=np.float32)
    Wqkv = np.asarray(Wqkv, dtype=np.float32)
    bqkv = np.asarray(bqkv, dtype=np.float32)
    Wout = np.asarray(Wout, dtype=np.float32)
    bout = np.asarray(bout, dtype=np.float32)
    assert x.shape == (B, T, C), x.shape

    if 'nc' not in _CACHED:
        _CACHED['nc'] = _build_program()
    nc = _CACHED['nc']

    in_maps = _host_shards(x, Wqkv, bqkv, Wout)
    res = run_bass_kernel_spmd(nc, in_maps, core_ids=list(range(NCORES)))

    y = np.empty((B, T, C), np.float32)
    for b in range(B):
        y[b] = res.results[2 * b]["y"] + res.results[2 * b + 1]["y"] + bout
    return y


# revision 17
# speedup vs baseline: 1.0217x; 1.0217x over previous
"""Causal multi-head attention block (qkv proj + attention + out proj) on 8 TRN2 cores.

Problem: x[4,2048,1024] @ Wqkv[1024,3072] -> 16-head causal attention -> @ Wout.

Sharding: batch(4) x head-group(2) -> 8 cores. Core c handles batch c//2 and
heads (c%2)*8..(c%2)*8+8. Each core computes its 8 heads' attention and a
partial out-projection [2048,1024]; host sums the two head-group partials per
batch and adds bout.

v2 over the ~300us v1 baseline:
  - Q/K projections in fp8e4 DoubleRow (both operands fp8, 2 contraction
    chunks per instr): 2x PE throughput on mm_q/mm_k. Wq,Wk pre-scaled x32
    on host so fp8 operands use the e4m3 range (|psq| <~ 123 < 240).
  - Q,K stored fp8 (x32); mm_s in fp8 (runs at bf16 rate) with ROW TILING:
    head hb=0 lives at partitions 0-63, hb=1 at 64-127; adjacent mm_s
    instructions target disjoint row groups and execute CONCURRENTLY on the
    PE (measured 109ns vs 214ns per pair) -> 2x on mm_s. unit2() interleaves
    both heads of an hp pair to make the pairs adjacent.
  - exp ACT carries scale=1/8192 (undo 32*32 and the 1/sqrt(64)).
  - V path / ctx / out-proj stay bf16 (fp8 there fails the 2e-2 tolerance).
  - Causal slicing everywhere; masks shrink to one [128,128] triangle block.
  - Softmax denominator rides as V's 65th column through the ctx matmul.
  - Projection + out-proj matmuls are deadline-scheduled into the attention
    inner loop via a paced background closure queue.
  - PSUM: 2x[128,2,512] S (one per head of the pair) + 2x[128,512] proj +
    2x[65,512] ctx = 8 banks.
"""
import numpy as np

B, T, C = 4, 2048, 1024
H, HD = 16, 64
NCORES = 8
WSCALE = 32.0
EXP_SCALE = 1.0 / (WSCALE * WSCALE * 8.0)
DVE_EXP_EVERY = 0  # 0=off; else every Nth exp emission runs on DVE


def _fit_exp16_constants(lam, smax=3.65):
    """Degree-2 near-minimax fit q(v)~e^v on v=x*lam/16; exp(x*lam)=q(x)^16.
    Returns (c0, c1, c2) on raw x. Max rel err ~0.8% after ^16."""
    vmax = smax / 16.0
    v = np.linspace(-vmax, vmax, 20001)
    t = np.exp(v)
    w = 1.0 / t
    for _ in range(60):
        A = np.stack([np.ones_like(v), v, v * v], axis=1) * w[:, None]
        c = np.linalg.lstsq(A, t * w, rcond=None)[0]
        relerr = np.abs((c[0] + c[1] * v + c[2] * v * v) / t - 1.0)
        w = w * (1.0 + 2.0 * relerr / max(relerr.max(), 1e-12))
        w /= w.max()
    k = lam / 16.0
    return float(c[0]), float(c[1] * k), float(c[2] * k * k)


def _register_exp16():
    """Custom DVE op out = ((c2 x + c1) x + c0)^16 ~ exp(x*EXP_SCALE).
    8 ALU stages (v3 budget). Appended to concourse.dve_ops at runtime."""
    import concourse.dve_ops as dops
    from concourse.dve_spec import Spec, Src0, C0, C1, C2, lower, sq, _has_src1
    from concourse.dve_uop import DveOpSpec

    name = "EXP_PWR16_ANT"
    for op in dops.OPS:
        if op.name == name:
            return op
    body = sq(sq(sq(sq((C2 * Src0 + C1) * Src0 + C0))))

    def _ref(in0, in1, s0, s1, imm2):
        q = (imm2 * in0 + s1) * in0 + s0
        return q ** 16

    spec = Spec(body=body, reference=_ref)
    opcode = dops._CUSTOM_DVE_ROW_BASE + len(dops.OPS)
    shas = {}
    for ver in ("v3", "v4"):
        try:
            compiled = DveOpSpec(name=name, opcode=opcode,
                                 uops=lower(spec, ver=ver),
                                 rd1_en=_has_src1(spec))
            shas[ver] = compiled.sha(ver)
        except Exception:
            pass
    op = dops.DveOp(name, spec, subdim=False, uops_sha=shas)
    dops.OPS.append(op)
    dops.CUSTOM_DVE_SPECS[name] = spec
    dops._SUB_OPCODE_FOR_NAME[name] = opcode
    return op


def _build_program():
    import concourse.bacc as bacc
    import concourse.tile as tile
    from concourse import mybir

    dtf = mybir.dt.float32
    dtr = mybir.dt.float32r
    dtb = mybir.dt.bfloat16
    dt8 = mybir.dt.float8e4
    EXP = mybir.ActivationFunctionType.Exp
    MULT = mybir.AluOpType.mult
    DR = mybir.MatmulPerfMode.DoubleRow

    exp16 = _register_exp16()
    ec0, ec1, ec2 = _fit_exp16_constants(EXP_SCALE)
    expctr = {'i': 0}

    nc = bacc.Bacc('TRN2', target_bir_lowering=False, debug=False)
    xt8_d = nc.dram_tensor("xt8", [1024, 2048], dt8, kind="ExternalInput").ap()
    xt_d = nc.dram_tensor("xt", [1024, 2048], dtb, kind="ExternalInput").ap()
    wqk_d = nc.dram_tensor("wqk", [1024, 1024], dt8, kind="ExternalInput").ap()
    wv_d = nc.dram_tensor("wv", [1024, 512], dtb, kind="ExternalInput").ap()
    wout_d = nc.dram_tensor("wout", [512, 1024], dtb, kind="ExternalInput").ap()
    mask_d = nc.dram_tensor("mask", [4, 128, 512], dtb, kind="ExternalInput").ap()
    y_d = nc.dram_tensor("y", [2048, 1024], dtf, kind="ExternalOutput").ap()

    with tile.TileContext(nc) as tc:
        with tc.tile_pool(name="ps_s", bufs=2, space="PSUM") as ps_s, \
             tc.tile_pool(name="ps", bufs=2, space="PSUM") as ps, \
             tc.tile_pool(name="ps_ctx", bufs=2, space="PSUM") as ps_ctx, \
             tc.tile_pool(name="const", bufs=1) as const, \
             tc.tile_pool(name="xt_p", bufs=16) as xt_p, \
             tc.tile_pool(name="xt8_p", bufs=8) as xt8_p, \
             tc.tile_pool(name="qt_p", bufs=8) as qt_p, \
             tc.tile_pool(name="exp_p", bufs=8) as exp_p, \
             tc.tile_pool(name="cn_p", bufs=16) as cn_p, \
             tc.tile_pool(name="row_p", bufs=4) as row_p, \
             tc.tile_pool(name="rcp_p", bufs=2) as rcp_p, \
             tc.tile_pool(name="y_p", bufs=2) as y_p:

            # ---- constants / weights ----
            # wqk_sb[p, kp, i, oc, c]: fp8 DR layout; rows 256*kp+128*i+p of
            # Wqk, cols oc*128+c. oc 0-3 Q, 4-7 K.
            wqk_sb = const.tile([128, 4, 2, 8, 128], dt8)
            wv_sb = const.tile([128, 8, 512], dtb)       # [p, kc, n]
            wout_sb = const.tile([128, 4, 2, 512], dtb)  # [p, hp, oc, c]
            masks = const.tile([128, 4, 512], dtb)
            ones_f32 = const.tile([1, 128], dtf)
            ones_t = const.tile([1, 128], dtr)
            kt_store = const.tile([128, 4, 4, 512], dt8)  # [p, hp, tt, t]
            v_all = const.tile([128, 16, 8, 65], dtb)     # [p, kt, h, d|1]

            nc.vector.memset(v_all[:, :, :, 64:65], 1.0)
            nc.vector.memset(ones_f32[:], 1.0)
            nc.vector.tensor_copy(ones_t[:], ones_f32[:])

            qts = {}   # tt -> [4 qt tiles (fp8)]
            xts = {}   # tt -> [8 bf16 xt tiles]
            x8s = {}   # tt -> [4 fp8 xt DR tiles]
            cns = {}   # qt -> [4 cn tiles]

            def emit_xt_dma(tt):
                ts = []
                for kc in range(8):
                    t_ = xt_p.tile([128, 512], dtb, tag="xt")
                    nc.sync.dma_start(
                        t_[:], xt_d[kc * 128:(kc + 1) * 128,
                                    tt * 512:(tt + 1) * 512])
                    ts.append(t_)
                xts[tt] = ts
                t8s = []
                for kp in range(4):
                    t_ = xt8_p.tile([128, 2, 512], dt8, tag="xt8")
                    nc.scalar.dma_start(
                        t_[:], xt8_d[kp * 256:(kp + 1) * 256,
                                     tt * 512:(tt + 1) * 512]
                        .rearrange("(i p) t -> p i t", p=128))
                    t8s.append(t_)
                x8s[tt] = t8s

            # DMA order: Q-half wqk chunks (sync) + xt0 feed the first Q
            # matmuls ASAP; K-half wqk rides the gpsimd queue.
            xts[0] = []
            x8s[0] = []
            for kp in range(4):
                nc.sync.dma_start(
                    wqk_sb[:, kp, :, 0:4, :],
                    wqk_d[kp * 256:(kp + 1) * 256, 0:512]
                    .rearrange("(i p) (oc c) -> p i oc c", p=128, c=128))
                t_ = xt8_p.tile([128, 2, 512], dt8, tag="xt8", name="xt80")
                nc.scalar.dma_start(
                    t_[:], xt8_d[kp * 256:(kp + 1) * 256, 0:512]
                    .rearrange("(i p) t -> p i t", p=128))
                x8s[0].append(t_)
                nc.gpsimd.dma_start(
                    wqk_sb[:, kp, :, 4:8, :],
                    wqk_d[kp * 256:(kp + 1) * 256, 512:1024]
                    .rearrange("(i p) (oc c) -> p i oc c", p=128, c=128))
            for kc in range(8):
                t_ = xt_p.tile([128, 512], dtb, tag="xt", name="xt0")
                nc.scalar.dma_start(
                    t_[:], xt_d[kc * 128:(kc + 1) * 128, 0:512])
                xts[0].append(t_)
            nc.sync.dma_start(wv_sb[:],
                              wv_d.rearrange("(kc p) n -> p kc n", p=128))
            nc.sync.dma_start(masks[:], mask_d.rearrange("n p f -> p n f"))
            emit_xt_dma(1)
            nc.sync.dma_start(wout_sb[:],
                              wout_d.rearrange("(hp p) (oc c) -> p hp oc c",
                                               p=128, c=512))

            def proj_closures(tt):
                cl = []
                qts[tt] = [None] * 4
                state = {}

                def q_mm(j, kp):
                    if kp == 0:
                        state[('q', j)] = ps.tile([128, 512], dtf, tag="ps", name="psq")
                    psq = state[('q', j)]
                    nc.tensor.matmul(psq[:], wqk_sb[:, kp, :, j, :],
                                     x8s[tt][kp][:], start=(kp == 0),
                                     stop=(kp == 3),
                                     perf_mode=DR).annotate('mm_q')
                    if kp == 3:
                        qt_t = qt_p.tile([128, 512], dt8, tag="qt")
                        nc.vector.tensor_copy(qt_t[:], psq[:])
                        qts[tt][j] = qt_t

                def k_mm(j, kp):
                    if kp == 0:
                        state[('k', j)] = ps.tile([128, 512], dtf, tag="ps", name="psk")
                    psk = state[('k', j)]
                    nc.tensor.matmul(psk[:], wqk_sb[:, kp, :, 4 + j, :],
                                     x8s[tt][kp][:], start=(kp == 0),
                                     stop=(kp == 3),
                                     perf_mode=DR).annotate('mm_k')
                    if kp == 3:
                        nc.vector.tensor_copy(kt_store[:, j, tt, :], psk[:])

                def v_mm(sub, kc):
                    if kc == 0:
                        state[('v', sub)] = ps.tile([128, 512], dtf, tag="ps", name="psv")
                    psv = state[('v', sub)]
                    nc.tensor.matmul(psv[:],
                                     xts[tt][kc][:, sub * 128:(sub + 1) * 128],
                                     wv_sb[:, kc, :], start=(kc == 0),
                                     stop=(kc == 7)).annotate('mm_v')
                    if kc == 7:
                        vt = tt * 4 + sub
                        nc.vector.tensor_copy(
                            v_all[:, vt, :, 0:64],
                            psv[:].rearrange("p (h d) -> p h d", h=8))

                for j in range(4):
                    for kp in range(4):
                        cl.append(lambda j=j, kp=kp: q_mm(j, kp))
                for j in range(4):
                    for kp in range(4):
                        cl.append(lambda j=j, kp=kp: k_mm(j, kp))
                for sub in range(4):
                    for kc in range(8):
                        cl.append(lambda s=sub, kc=kc: v_mm(s, kc))
                return cl

            def y_closures(qt):
                cl = []
                state = {}

                def y_mm(mi, oc, hp):
                    if hp == 0:
                        state[(mi, oc)] = ps.tile([128, 512], dtf, tag="ps", name="psy")
                    psy = state[(mi, oc)]
                    nc.tensor.matmul(psy[:],
                                     cns[qt][hp][:, mi * 128:(mi + 1) * 128],
                                     wout_sb[:, hp, oc, :],
                                     start=(hp == 0), stop=(hp == 3)).annotate('mm_y')
                    if hp == 3:
                        y_sb = y_p.tile([128, 512], dtf, tag="y")
                        nc.vector.tensor_copy(y_sb[:], psy[:])
                        nc.sync.dma_start(
                            y_d[qt * 512 + mi * 128: qt * 512 + (mi + 1) * 128,
                                oc * 512:(oc + 1) * 512],
                            y_sb[:])

                for mi in range(4):
                    for oc in range(2):
                        for hp in range(4):
                            cl.append(lambda m=mi, o=oc, h=hp: y_mm(m, o, h))
                return cl

            def unit2(qt, hp, cn_t, pull):
                """Both hb heads of pair hp, interleaved so mm_s for hb=0
                (rows 0-63) and hb=1 (rows 64-127) are pc-adjacent and run
                concurrently on disjoint PE row groups."""
                n_kt = 4 * qt + 4
                np_ = n_kt // 2
                ctx = [ps_ctx.tile([65, 512], dtf, tag="ctx", name=f"ctx{hb}")
                       for hb in (0, 1)]
                pending = []

                def off(kt):
                    di = kt - 4 * qt
                    return 128 * di if di > 0 else 0

                def emit_ctx(j, exs):
                    for hb in (0, 1):
                        for t in (0, 1):
                            kt = 2 * j + t
                            o = off(kt)
                            nc.tensor.matmul(ctx[hb][:, o:512],
                                             v_all[:, kt, 2 * hp + hb, :],
                                             exs[hb][:, t, o:512],
                                             start=(kt == 0),
                                             stop=(kt == n_kt - 1)).annotate('mm_ctx')

                for j in range(np_):
                    sps = [ps_s.tile([128, 2, 512], dtf, tag="s", name=f"sp{hb}")
                           for hb in (0, 1)]
                    for t in (0, 1):
                        kt = 2 * j + t
                        ktt, kj = kt // 4, kt % 4
                        o = off(kt)
                        for hb in (0, 1):
                            pb = hb * 64
                            nc.tensor.matmul(
                                sps[hb][:, t, o:512],
                                kt_store[pb:pb + 64, hp, ktt,
                                         kj * 128:(kj + 1) * 128],
                                qts[qt][hp][pb:pb + 64, o:512],
                                start=True, stop=True).annotate('mm_s')
                    pull(4)
                    exs = []
                    for hb in (0, 1):
                        ex = exp_p.tile([128, 2, 512], dtb, tag="exp")
                        expctr['i'] += 1
                        on_dve = DVE_EXP_EVERY and (expctr['i'] % DVE_EXP_EVERY) == 0

                        def _exp(dst, src):
                            if on_dve:
                                nc.vector._custom_dve(exp16, out=dst, in0=src,
                                                      s0=ec0, s1=ec1, imm2=ec2)
                            else:
                                nc.scalar.activation(dst, src, EXP,
                                                     scale=EXP_SCALE)
                        if off(2 * j) >= 256:
                            for t in (0, 1):
                                o = off(2 * j + t)
                                _exp(ex[:, t, o:512], sps[hb][:, t, o:512])
                        else:
                            _exp(ex[:], sps[hb][:])
                        for t in (0, 1):
                            kt = 2 * j + t
                            di = kt - 4 * qt
                            if 0 <= di < 4:
                                o, w = 128 * di, 128 * (di + 1)
                                nc.vector.tensor_tensor(ex[:, t, o:w],
                                                        ex[:, t, o:w],
                                                        masks[:, di, o:w], MULT)
                        exs.append(ex)
                    pending.append((j, exs))
                    if len(pending) > 3:
                        emit_ctx(*pending.pop(0))
                        pull(2)
                while pending:
                    emit_ctx(*pending.pop(0))
                    pull(2)
                # normalize both heads' halves of cn
                for hb in (0, 1):
                    pb = hb * 64
                    if qt == 3 and hp >= 2:
                        # final units feed the epilogue now: broadcast on PE
                        row = row_p.tile([1, 512], dtr, tag="row")
                        nc.vector.tensor_copy(row[:], ctx[hb][64:65, :])
                        bc = ps.tile([128, 512], dtf, tag="ps", name="bc")
                        nc.tensor.matmul(bc[:], ones_t[:], row[:],
                                         start=True, stop=True).annotate('mm_bc')
                        rcp = rcp_p.tile([64, 512], dtf, tag="rcp", name="rcp")
                        nc.vector.reciprocal_approx_fast(rcp[:], bc[0:64, :])
                        nc.vector.tensor_tensor(cn_t[pb:pb + 64, :],
                                                ctx[hb][0:64, :], rcp[:], MULT)
                        continue
                    row = row_p.tile([1, 512], dtf, tag="row")
                    nc.vector.tensor_copy(row[:], ctx[hb][64:65, :])
                    rrow = row_p.tile([1, 512], dtf, tag="row", name="rrow")
                    nc.vector.reciprocal_approx_fast(rrow[:], row[:])
                    pull(1)
                    rcp = rcp_p.tile([64, 512], dtf, tag="rcp")
                    nc.gpsimd.partition_broadcast(rcp[:], rrow[:])
                    nc.vector.tensor_tensor(cn_t[pb:pb + 64, :],
                                            ctx[hb][0:64, :], rcp[:], MULT)

            def make_puller(items, total_slots):
                st = {'i': 0, 'slot': 0}
                n = len(items)

                def pull(k):
                    st['slot'] += k
                    if total_slots > 0:
                        target = min(n, (n * st['slot'] + total_slots - 1)
                                     // total_slots)
                    else:
                        target = n
                    # burst to a multiple of 8: longer same-type matmul runs
                    # keep the PE weight double-buffer streaming
                    if st['i'] < target:
                        target = min(n, ((target + 7) // 8) * 8)
                    while st['i'] < target:
                        items[st['i']]()
                        st['i'] += 1

                def drain():
                    while st['i'] < n:
                        items[st['i']]()
                        st['i'] += 1
                return pull, drain

            # ---- prologue: tt=0 projections ----
            warm = row_p.tile([1, 1], dtf, tag="row", name="warm")
            nc.scalar.activation(warm[:], v_all[0:1, 0, 0, 64:65], EXP)
            for c in proj_closures(0):
                c()

            # ---- main loop ----
            for qt in range(4):
                Bq = []
                if qt < 2:
                    emit_xt_dma(qt + 2)
                if qt < 3:
                    Bq += proj_closures(qt + 1)
                if qt == 3:
                    Bq += (y_closures(0) + y_closures(1) + y_closures(2)
                           + y_closures(3))
                np_ = (4 * qt + 4) // 2
                total_slots = 4 * (8 * np_ + 2)
                pull, drain = make_puller(Bq, total_slots)
                # pre-register all four cn tiles so late-paced y(3)
                # closures can reference cns[3][hp] before unit2(hp) runs;
                # the tile framework orders mm_y after the cn writes.
                cns[qt] = [cn_p.tile([128, 512], dtb, tag="cn", name=f"cn{hp}")
                           for hp in range(4)]
                for hp in range(4):
                    unit2(qt, hp, cns[qt][hp], pull)
                drain()
    nc.compile()
    return nc


def _host_shards(x, Wqkv, bqkv, Wout):
    import ml_dtypes
    mask = np.zeros((4, 128, 512), np.float32)
    qq = np.arange(512)[None, :]
    kk = np.arange(128)[:, None]
    for di in range(4):
        mask[di] = (kk + di * 128 <= qq)
    mask = mask.astype(ml_dtypes.bfloat16)

    assert not np.any(bqkv), "kernel assumes zero qkv bias"

    in_maps = []
    for c in range(NCORES):
        b, hg = c // 2, c % 2
        s = hg * 512
        xt_f = np.ascontiguousarray(x[b].T)
        xt8 = xt_f.astype(ml_dtypes.float8_e4m3)
        xt = xt_f.astype(ml_dtypes.bfloat16)
        wqk = np.ascontiguousarray(
            np.concatenate([Wqkv[:, s:s + 512] * WSCALE,
                            Wqkv[:, 1024 + s:1024 + s + 512] * WSCALE],
                           axis=1)).astype(ml_dtypes.float8_e4m3)
        wv = np.ascontiguousarray(Wqkv[:, 2048 + s:2048 + s + 512]).astype(ml_dtypes.bfloat16)
        wout = np.ascontiguousarray(Wout[s:s + 512, :]).astype(ml_dtypes.bfloat16)
        in_maps.append({"xt8": xt8, "xt": xt, "wqk": wqk, "wv": wv,
                        "wout": wout, "mask": mask})
    return in_maps


_CACHED = {}


def kernel(x, Wqkv, bqkv, Wout, bout):
    from concourse.bass_utils import run_bass_kernel_spmd

    x = np.asarray(x, dtype=np.float32)
    Wqkv = np.asarray(Wqkv, dtype=np.float32)
    bqkv = np.asarray(bqkv, dtype=np.float32)
    Wout = np.asarray(Wout, dtype=np.float32)
    bout = np.asarray(bout, dtype=np.float32)
    assert x.shape == (B, T, C), x.shape

    if 'nc' not in _CACHED:
        _CACHED['nc'] = _build_program()
    nc = _CACHED['nc']

    in_maps = _host_shards(x, Wqkv, bqkv, Wout)
    res = run_bass_kernel_spmd(nc, in_maps, core_ids=list(range(NCORES)))

    y = np.empty((B, T, C), np.float32)
    for b in range(B):
        y[b] = res.results[2 * b]["y"] + res.results[2 * b + 1]["y"] + bout
    return y


# revision 18
# speedup vs baseline: 1.0303x; 1.0084x over previous
"""Causal multi-head attention block (qkv proj + attention + out proj) on 8 TRN2 cores.

Problem: x[4,2048,1024] @ Wqkv[1024,3072] -> 16-head causal attention -> @ Wout.

Sharding: batch(4) x head-group(2) -> 8 cores. Core c handles batch c//2 and
heads (c%2)*8..(c%2)*8+8. Each core computes its 8 heads' attention and a
partial out-projection [2048,1024]; host sums the two head-group partials per
batch and adds bout.

v2 over the ~300us v1 baseline:
  - Q/K projections in fp8e4 DoubleRow (both operands fp8, 2 contraction
    chunks per instr): 2x PE throughput on mm_q/mm_k. Wq,Wk pre-scaled x32
    on host so fp8 operands use the e4m3 range (|psq| <~ 123 < 240).
  - Q,K stored fp8 (x32); mm_s in fp8 (runs at bf16 rate) with ROW TILING:
    head hb=0 lives at partitions 0-63, hb=1 at 64-127; adjacent mm_s
    instructions target disjoint row groups and execute CONCURRENTLY on the
    PE (measured 109ns vs 214ns per pair) -> 2x on mm_s. unit2() interleaves
    both heads of an hp pair to make the pairs adjacent.
  - exp ACT carries scale=1/8192 (undo 32*32 and the 1/sqrt(64)).
  - V path / ctx / out-proj stay bf16 (fp8 there fails the 2e-2 tolerance).
  - Causal slicing everywhere; masks shrink to one [128,128] triangle block.
  - Softmax denominator rides as V's 65th column through the ctx matmul.
  - Projection + out-proj matmuls are deadline-scheduled into the attention
    inner loop via a paced background closure queue.
  - PSUM: 2x[128,2,512] S (one per head of the pair) + 2x[128,512] proj +
    2x[65,512] ctx = 8 banks.
"""
import numpy as np

B, T, C = 4, 2048, 1024
H, HD = 16, 64
NCORES = 8
WSCALE = 32.0
EXP_SCALE = 1.0 / (WSCALE * WSCALE * 8.0)
DVE_EXP_EVERY = 0  # 0=off; else every Nth exp emission runs on DVE


def _fit_exp16_constants(lam, smax=3.65):
    """Degree-2 near-minimax fit q(v)~e^v on v=x*lam/16; exp(x*lam)=q(x)^16.
    Returns (c0, c1, c2) on raw x. Max rel err ~0.8% after ^16."""
    vmax = smax / 16.0
    v = np.linspace(-vmax, vmax, 20001)
    t = np.exp(v)
    w = 1.0 / t
    for _ in range(60):
        A = np.stack([np.ones_like(v), v, v * v], axis=1) * w[:, None]
        c = np.linalg.lstsq(A, t * w, rcond=None)[0]
        relerr = np.abs((c[0] + c[1] * v + c[2] * v * v) / t - 1.0)
        w = w * (1.0 + 2.0 * relerr / max(relerr.max(), 1e-12))
        w /= w.max()
    k = lam / 16.0
    return float(c[0]), float(c[1] * k), float(c[2] * k * k)


def _register_exp16():
    """Custom DVE op out = ((c2 x + c1) x + c0)^16 ~ exp(x*EXP_SCALE).
    8 ALU stages (v3 budget). Appended to concourse.dve_ops at runtime."""
    import concourse.dve_ops as dops
    from concourse.dve_spec import Spec, Src0, C0, C1, C2, lower, sq, _has_src1
    from concourse.dve_uop import DveOpSpec

    name = "EXP_PWR16_ANT"
    for op in dops.OPS:
        if op.name == name:
            return op
    body = sq(sq(sq(sq((C2 * Src0 + C1) * Src0 + C0))))

    def _ref(in0, in1, s0, s1, imm2):
        q = (imm2 * in0 + s1) * in0 + s0
        return q ** 16

    spec = Spec(body=body, reference=_ref)
    opcode = dops._CUSTOM_DVE_ROW_BASE + len(dops.OPS)
    shas = {}
    for ver in ("v3", "v4"):
        try:
            compiled = DveOpSpec(name=name, opcode=opcode,
                                 uops=lower(spec, ver=ver),
                                 rd1_en=_has_src1(spec))
            shas[ver] = compiled.sha(ver)
        except Exception:
            pass
    op = dops.DveOp(name, spec, subdim=False, uops_sha=shas)
    dops.OPS.append(op)
    dops.CUSTOM_DVE_SPECS[name] = spec
    dops._SUB_OPCODE_FOR_NAME[name] = opcode
    return op


def _build_program():
    import concourse.bacc as bacc
    import concourse.tile as tile
    from concourse import mybir

    dtf = mybir.dt.float32
    dtr = mybir.dt.float32r
    dtb = mybir.dt.bfloat16
    dt8 = mybir.dt.float8e4
    EXP = mybir.ActivationFunctionType.Exp
    MULT = mybir.AluOpType.mult
    DR = mybir.MatmulPerfMode.DoubleRow

    exp16 = _register_exp16()
    ec0, ec1, ec2 = _fit_exp16_constants(EXP_SCALE)
    expctr = {'i': 0}

    nc = bacc.Bacc('TRN2', target_bir_lowering=False, debug=False)
    xt8_d = nc.dram_tensor("xt8", [1024, 2048], dt8, kind="ExternalInput").ap()
    xt_d = nc.dram_tensor("xt", [1024, 2048], dtb, kind="ExternalInput").ap()
    wqk_d = nc.dram_tensor("wqk", [1024, 1024], dt8, kind="ExternalInput").ap()
    wv_d = nc.dram_tensor("wv", [1024, 512], dtb, kind="ExternalInput").ap()
    wout_d = nc.dram_tensor("wout", [512, 1024], dtb, kind="ExternalInput").ap()
    mask_d = nc.dram_tensor("mask", [4, 128, 512], dtb, kind="ExternalInput").ap()
    y_d = nc.dram_tensor("y", [2048, 1024], dtf, kind="ExternalOutput").ap()

    with tile.TileContext(nc) as tc:
        with tc.tile_pool(name="ps_s", bufs=2, space="PSUM") as ps_s, \
             tc.tile_pool(name="ps", bufs=2, space="PSUM") as ps, \
             tc.tile_pool(name="ps_ctx", bufs=2, space="PSUM") as ps_ctx, \
             tc.tile_pool(name="const", bufs=1) as const, \
             tc.tile_pool(name="xt_p", bufs=16) as xt_p, \
             tc.tile_pool(name="xt8_p", bufs=8) as xt8_p, \
             tc.tile_pool(name="qt_p", bufs=8) as qt_p, \
             tc.tile_pool(name="exp_p", bufs=8) as exp_p, \
             tc.tile_pool(name="cn_p", bufs=16) as cn_p, \
             tc.tile_pool(name="row_p", bufs=4) as row_p, \
             tc.tile_pool(name="rcp_p", bufs=2) as rcp_p, \
             tc.tile_pool(name="y_p", bufs=2) as y_p:

            # ---- constants / weights ----
            # wqk_sb[p, kp, i, oc, c]: fp8 DR layout; rows 256*kp+128*i+p of
            # Wqk, cols oc*128+c. oc 0-3 Q, 4-7 K.
            wqk_sb = const.tile([128, 4, 2, 8, 128], dt8)
            wv_sb = const.tile([128, 8, 512], dtb)       # [p, kc, n]
            wout_sb = const.tile([128, 4, 2, 512], dtb)  # [p, hp, oc, c]
            masks = const.tile([128, 4, 512], dtb)
            ones_f32 = const.tile([1, 128], dtf)
            ones_t = const.tile([1, 128], dtr)
            kt_store = const.tile([128, 4, 4, 512], dt8)  # [p, hp, tt, t]
            v_all = const.tile([128, 16, 8, 65], dtb)     # [p, kt, h, d|1]

            nc.vector.memset(v_all[:, :, :, 64:65], 1.0)
            nc.vector.memset(ones_f32[:], 1.0)
            nc.vector.tensor_copy(ones_t[:], ones_f32[:])

            qts = {}   # tt -> [4 qt tiles (fp8)]
            xts = {}   # tt -> [8 bf16 xt tiles]
            x8s = {}   # tt -> [4 fp8 xt DR tiles]
            cns = {}   # qt -> [4 cn tiles]

            def emit_xt_dma(tt):
                ts = []
                for kc in range(8):
                    t_ = xt_p.tile([128, 512], dtb, tag="xt")
                    nc.sync.dma_start(
                        t_[:], xt_d[kc * 128:(kc + 1) * 128,
                                    tt * 512:(tt + 1) * 512])
                    ts.append(t_)
                xts[tt] = ts
                t8s = []
                for kp in range(4):
                    t_ = xt8_p.tile([128, 2, 512], dt8, tag="xt8")
                    nc.gpsimd.dma_start(
                        t_[:], xt8_d[kp * 256:(kp + 1) * 256,
                                     tt * 512:(tt + 1) * 512]
                        .rearrange("(i p) t -> p i t", p=128))
                    t8s.append(t_)
                x8s[tt] = t8s

            # DMA order: Q-half wqk chunks (sync) + xt0 feed the first Q
            # matmuls ASAP; K-half wqk rides the gpsimd queue.
            xts[0] = []
            x8s[0] = []
            for kp in range(4):
                nc.sync.dma_start(
                    wqk_sb[:, kp, :, 0:4, :],
                    wqk_d[kp * 256:(kp + 1) * 256, 0:512]
                    .rearrange("(i p) (oc c) -> p i oc c", p=128, c=128))
                t_ = xt8_p.tile([128, 2, 512], dt8, tag="xt8", name="xt80")
                nc.scalar.dma_start(
                    t_[:], xt8_d[kp * 256:(kp + 1) * 256, 0:512]
                    .rearrange("(i p) t -> p i t", p=128))
                x8s[0].append(t_)
                nc.gpsimd.dma_start(
                    wqk_sb[:, kp, :, 4:8, :],
                    wqk_d[kp * 256:(kp + 1) * 256, 512:1024]
                    .rearrange("(i p) (oc c) -> p i oc c", p=128, c=128))
            for kc in range(8):
                t_ = xt_p.tile([128, 512], dtb, tag="xt", name="xt0")
                nc.scalar.dma_start(
                    t_[:], xt_d[kc * 128:(kc + 1) * 128, 0:512])
                xts[0].append(t_)
            nc.sync.dma_start(wv_sb[:],
                              wv_d.rearrange("(kc p) n -> p kc n", p=128))
            nc.sync.dma_start(masks[:], mask_d.rearrange("n p f -> p n f"))
            emit_xt_dma(1)
            nc.sync.dma_start(wout_sb[:],
                              wout_d.rearrange("(hp p) (oc c) -> p hp oc c",
                                               p=128, c=512))

            def proj_closures(tt):
                cl = []
                qts[tt] = [None] * 4
                state = {}

                def q_mm(j, kp):
                    if kp == 0:
                        state[('q', j)] = ps.tile([128, 512], dtf, tag="ps", name="psq")
                    psq = state[('q', j)]
                    nc.tensor.matmul(psq[:], wqk_sb[:, kp, :, j, :],
                                     x8s[tt][kp][:], start=(kp == 0),
                                     stop=(kp == 3),
                                     perf_mode=DR).annotate('mm_q')
                    if kp == 3:
                        qt_t = qt_p.tile([128, 512], dt8, tag="qt")
                        nc.vector.tensor_copy(qt_t[:], psq[:])
                        qts[tt][j] = qt_t

                def k_mm(j, kp):
                    if kp == 0:
                        state[('k', j)] = ps.tile([128, 512], dtf, tag="ps", name="psk")
                    psk = state[('k', j)]
                    nc.tensor.matmul(psk[:], wqk_sb[:, kp, :, 4 + j, :],
                                     x8s[tt][kp][:], start=(kp == 0),
                                     stop=(kp == 3),
                                     perf_mode=DR).annotate('mm_k')
                    if kp == 3:
                        nc.vector.tensor_copy(kt_store[:, j, tt, :], psk[:])

                def v_mm(sub, kc):
                    if kc == 0:
                        state[('v', sub)] = ps.tile([128, 512], dtf, tag="ps", name="psv")
                    psv = state[('v', sub)]
                    nc.tensor.matmul(psv[:],
                                     xts[tt][kc][:, sub * 128:(sub + 1) * 128],
                                     wv_sb[:, kc, :], start=(kc == 0),
                                     stop=(kc == 7)).annotate('mm_v')
                    if kc == 7:
                        vt = tt * 4 + sub
                        nc.vector.tensor_copy(
                            v_all[:, vt, :, 0:64],
                            psv[:].rearrange("p (h d) -> p h d", h=8))

                for j in range(4):
                    for kp in range(4):
                        cl.append(lambda j=j, kp=kp: q_mm(j, kp))
                for j in range(4):
                    for kp in range(4):
                        cl.append(lambda j=j, kp=kp: k_mm(j, kp))
                for sub in range(4):
                    for kc in range(8):
                        cl.append(lambda s=sub, kc=kc: v_mm(s, kc))
                return cl

            def y_closures(qt):
                cl = []
                state = {}

                def y_mm(mi, oc, hp):
                    if hp == 0:
                        state[(mi, oc)] = ps.tile([128, 512], dtf, tag="ps", name="psy")
                    psy = state[(mi, oc)]
                    nc.tensor.matmul(psy[:],
                                     cns[qt][hp][:, mi * 128:(mi + 1) * 128],
                                     wout_sb[:, hp, oc, :],
                                     start=(hp == 0), stop=(hp == 3)).annotate('mm_y')
                    if hp == 3:
                        y_sb = y_p.tile([128, 512], dtf, tag="y")
                        nc.vector.tensor_copy(y_sb[:], psy[:])
                        nc.sync.dma_start(
                            y_d[qt * 512 + mi * 128: qt * 512 + (mi + 1) * 128,
                                oc * 512:(oc + 1) * 512],
                            y_sb[:])

                for mi in range(4):
                    for oc in range(2):
                        for hp in range(4):
                            cl.append(lambda m=mi, o=oc, h=hp: y_mm(m, o, h))
                return cl

            def unit2(qt, hp, cn_t, pull):
                """Both hb heads of pair hp, interleaved so mm_s for hb=0
                (rows 0-63) and hb=1 (rows 64-127) are pc-adjacent and run
                concurrently on disjoint PE row groups."""
                n_kt = 4 * qt + 4
                np_ = n_kt // 2
                ctx = [ps_ctx.tile([65, 512], dtf, tag="ctx", name=f"ctx{hb}")
                       for hb in (0, 1)]
                pending = []

                def off(kt):
                    di = kt - 4 * qt
                    return 128 * di if di > 0 else 0

                def emit_ctx(j, exs):
                    for hb in (0, 1):
                        for t in (0, 1):
                            kt = 2 * j + t
                            o = off(kt)
                            nc.tensor.matmul(ctx[hb][:, o:512],
                                             v_all[:, kt, 2 * hp + hb, :],
                                             exs[hb][:, t, o:512],
                                             start=(kt == 0),
                                             stop=(kt == n_kt - 1)).annotate('mm_ctx')

                for j in range(np_):
                    sps = [ps_s.tile([128, 2, 512], dtf, tag="s", name=f"sp{hb}")
                           for hb in (0, 1)]
                    for t in (0, 1):
                        kt = 2 * j + t
                        ktt, kj = kt // 4, kt % 4
                        o = off(kt)
                        for hb in (0, 1):
                            pb = hb * 64
                            nc.tensor.matmul(
                                sps[hb][:, t, o:512],
                                kt_store[pb:pb + 64, hp, ktt,
                                         kj * 128:(kj + 1) * 128],
                                qts[qt][hp][pb:pb + 64, o:512],
                                start=True, stop=True).annotate('mm_s')
                    pull(4)
                    exs = []
                    for hb in (0, 1):
                        ex = exp_p.tile([128, 2, 512], dtb, tag="exp")
                        expctr['i'] += 1
                        on_dve = DVE_EXP_EVERY and (expctr['i'] % DVE_EXP_EVERY) == 0

                        def _exp(dst, src):
                            if on_dve:
                                nc.vector._custom_dve(exp16, out=dst, in0=src,
                                                      s0=ec0, s1=ec1, imm2=ec2)
                            else:
                                nc.scalar.activation(dst, src, EXP,
                                                     scale=EXP_SCALE)
                        if off(2 * j) >= 256:
                            for t in (0, 1):
                                o = off(2 * j + t)
                                _exp(ex[:, t, o:512], sps[hb][:, t, o:512])
                        else:
                            _exp(ex[:], sps[hb][:])
                        for t in (0, 1):
                            kt = 2 * j + t
                            di = kt - 4 * qt
                            if 0 <= di < 4:
                                o, w = 128 * di, 128 * (di + 1)
                                nc.vector.tensor_tensor(ex[:, t, o:w],
                                                        ex[:, t, o:w],
                                                        masks[:, di, o:w], MULT)
                        exs.append(ex)
                    pending.append((j, exs))
                    if len(pending) > 2:
                        emit_ctx(*pending.pop(0))
                        pull(2)
                while pending:
                    emit_ctx(*pending.pop(0))
                    pull(2)
                # normalize both heads' halves of cn
                for hb in (0, 1):
                    pb = hb * 64
                    if qt == 3 and hp >= 2:
                        # final units feed the epilogue now: broadcast on PE
                        row = row_p.tile([1, 512], dtr, tag="row")
                        nc.vector.tensor_copy(row[:], ctx[hb][64:65, :])
                        bc = ps.tile([128, 512], dtf, tag="ps", name="bc")
                        nc.tensor.matmul(bc[:], ones_t[:], row[:],
                                         start=True, stop=True).annotate('mm_bc')
                        rcp = rcp_p.tile([64, 512], dtf, tag="rcp", name="rcp")
                        nc.vector.reciprocal_approx_fast(rcp[:], bc[0:64, :])
                        nc.vector.tensor_tensor(cn_t[pb:pb + 64, :],
                                                ctx[hb][0:64, :], rcp[:], MULT)
                        continue
                    row = row_p.tile([1, 512], dtf, tag="row")
                    nc.vector.tensor_copy(row[:], ctx[hb][64:65, :])
                    rrow = row_p.tile([1, 512], dtf, tag="row", name="rrow")
                    nc.vector.reciprocal_approx_fast(rrow[:], row[:])
                    pull(1)
                    rcp = rcp_p.tile([64, 512], dtf, tag="rcp")
                    nc.gpsimd.partition_broadcast(rcp[:], rrow[:])
                    nc.vector.tensor_tensor(cn_t[pb:pb + 64, :],
                                            ctx[hb][0:64, :], rcp[:], MULT)

            def make_puller(items, total_slots):
                st = {'i': 0, 'slot': 0}
                n = len(items)

                def pull(k):
                    st['slot'] += k
                    if total_slots > 0:
                        target = min(n, (n * st['slot'] + total_slots - 1)
                                     // total_slots)
                    else:
                        target = n
                    # burst to a multiple of 8: longer same-type matmul runs
                    # keep the PE weight double-buffer streaming
                    if st['i'] < target:
                        target = min(n, ((target + 7) // 8) * 8)
                    while st['i'] < target:
                        items[st['i']]()
                        st['i'] += 1

                def drain():
                    while st['i'] < n:
                        items[st['i']]()
                        st['i'] += 1
                return pull, drain

            # ---- prologue: tt=0 projections ----
            warm = row_p.tile([1, 1], dtf, tag="row", name="warm")
            nc.scalar.activation(warm[:], v_all[0:1, 0, 0, 64:65], EXP)
            for c in proj_closures(0):
                c()

            # ---- main loop ----
            for qt in range(4):
                Bq = []
                if qt < 2:
                    emit_xt_dma(qt + 2)
                if qt < 3:
                    Bq += proj_closures(qt + 1)
                if qt == 3:
                    Bq += (y_closures(0) + y_closures(1) + y_closures(2)
                           + y_closures(3))
                np_ = (4 * qt + 4) // 2
                total_slots = 4 * (8 * np_ + 2)
                pull, drain = make_puller(Bq, total_slots)
                # pre-register all four cn tiles so late-paced y(3)
                # closures can reference cns[3][hp] before unit2(hp) runs;
                # the tile framework orders mm_y after the cn writes.
                cns[qt] = [cn_p.tile([128, 512], dtb, tag="cn", name=f"cn{hp}")
                           for hp in range(4)]
                for hp in range(4):
                    unit2(qt, hp, cns[qt][hp], pull)
                drain()
    nc.compile()
    return nc


def _host_shards(x, Wqkv, bqkv, Wout):
    import ml_dtypes
    mask = np.zeros((4, 128, 512), np.float32)
    qq = np.arange(512)[None, :]
    kk = np.arange(128)[:, None]
    for di in range(4):
        mask[di] = (kk + di * 128 <= qq)
    mask = mask.astype(ml_dtypes.bfloat16)

    assert not np.any(bqkv), "kernel assumes zero qkv bias"

    in_maps = []
    for c in range(NCORES):
        b, hg = c // 2, c % 2
        s = hg * 512
        xt_f = np.ascontiguousarray(x[b].T)
        xt8 = xt_f.astype(ml_dtypes.float8_e4m3)
        xt = xt_f.astype(ml_dtypes.bfloat16)
        wqk = np.ascontiguousarray(
            np.concatenate([Wqkv[:, s:s + 512] * WSCALE,
                            Wqkv[:, 1024 + s:1024 + s + 512] * WSCALE],
                           axis=1)).astype(ml_dtypes.float8_e4m3)
        wv = np.ascontiguousarray(Wqkv[:, 2048 + s:2048 + s + 512]).astype(ml_dtypes.bfloat16)
        wout = np.ascontiguousarray(Wout[s:s + 512, :]).astype(ml_dtypes.bfloat16)
        in_maps.append({"xt8": xt8, "xt": xt, "wqk": wqk, "wv": wv,
                        "wout": wout, "mask": mask})
    return in_maps


_CACHED = {}


def kernel(x, Wqkv, bqkv, Wout, bout):
    from concourse.bass_utils import run_bass_kernel_spmd

    x = np.asarray(x, dtype=np.float32)
    Wqkv = np.asarray(Wqkv, dtype=np.float32)
    bqkv = np.asarray(bqkv, dtype=np.float32)
    Wout = np.asarray(Wout, dtype=np.float32)
    bout = np.asarray(bout, dtype=np.float32)
    assert x.shape == (B, T, C), x.shape

    if 'nc' not in _CACHED:
        _CACHED['nc'] = _build_program()
    nc = _CACHED['nc']

    in_maps = _host_shards(x, Wqkv, bqkv, Wout)
    res = run_bass_kernel_spmd(nc, in_maps, core_ids=list(range(NCORES)))

    y = np.empty((B, T, C), np.float32)
    for b in range(B):
        y[b] = res.results[2 * b]["y"] + res.results[2 * b + 1]["y"] + bout
    return y


# revision 19
# speedup vs baseline: 1.0346x; 1.0041x over previous
"""Causal multi-head attention block (qkv proj + attention + out proj) on 8 TRN2 cores.

Problem: x[4,2048,1024] @ Wqkv[1024,3072] -> 16-head causal attention -> @ Wout.

Sharding: batch(4) x head-group(2) -> 8 cores. Core c handles batch c//2 and
heads (c%2)*8..(c%2)*8+8. Each core computes its 8 heads' attention and a
partial out-projection [2048,1024]; host sums the two head-group partials per
batch and adds bout.

v2 over the ~300us v1 baseline:
  - Q/K projections in fp8e4 DoubleRow (both operands fp8, 2 contraction
    chunks per instr): 2x PE throughput on mm_q/mm_k. Wq,Wk pre-scaled x32
    on host so fp8 operands use the e4m3 range (|psq| <~ 123 < 240).
  - Q,K stored fp8 (x32); mm_s in fp8 (runs at bf16 rate) with ROW TILING:
    head hb=0 lives at partitions 0-63, hb=1 at 64-127; adjacent mm_s
    instructions target disjoint row groups and execute CONCURRENTLY on the
    PE (measured 109ns vs 214ns per pair) -> 2x on mm_s. unit2() interleaves
    both heads of an hp pair to make the pairs adjacent.
  - exp ACT carries scale=1/8192 (undo 32*32 and the 1/sqrt(64)).
  - V path / ctx / out-proj stay bf16 (fp8 there fails the 2e-2 tolerance).
  - Causal slicing everywhere; masks shrink to one [128,128] triangle block.
  - Softmax denominator rides as V's 65th column through the ctx matmul.
  - Projection + out-proj matmuls are deadline-scheduled into the attention
    inner loop via a paced background closure queue.
  - PSUM: 2x[128,2,512] S (one per head of the pair) + 2x[128,512] proj +
    2x[65,512] ctx = 8 banks.
"""
import numpy as np

B, T, C = 4, 2048, 1024
H, HD = 16, 64
NCORES = 8
WSCALE = 32.0
EXP_SCALE = 1.0 / (WSCALE * WSCALE * 8.0)
DVE_EXP_EVERY = 0  # 0=off; else every Nth exp emission runs on DVE


def _fit_exp16_constants(lam, smax=3.65):
    """Degree-2 near-minimax fit q(v)~e^v on v=x*lam/16; exp(x*lam)=q(x)^16.
    Returns (c0, c1, c2) on raw x. Max rel err ~0.8% after ^16."""
    vmax = smax / 16.0
    v = np.linspace(-vmax, vmax, 20001)
    t = np.exp(v)
    w = 1.0 / t
    for _ in range(60):
        A = np.stack([np.ones_like(v), v, v * v], axis=1) * w[:, None]
        c = np.linalg.lstsq(A, t * w, rcond=None)[0]
        relerr = np.abs((c[0] + c[1] * v + c[2] * v * v) / t - 1.0)
        w = w * (1.0 + 2.0 * relerr / max(relerr.max(), 1e-12))
        w /= w.max()
    k = lam / 16.0
    return float(c[0]), float(c[1] * k), float(c[2] * k * k)


def _register_exp16():
    """Custom DVE op out = ((c2 x + c1) x + c0)^16 ~ exp(x*EXP_SCALE).
    8 ALU stages (v3 budget). Appended to concourse.dve_ops at runtime."""
    import concourse.dve_ops as dops
    from concourse.dve_spec import Spec, Src0, C0, C1, C2, lower, sq, _has_src1
    from concourse.dve_uop import DveOpSpec

    name = "EXP_PWR16_ANT"
    for op in dops.OPS:
        if op.name == name:
            return op
    body = sq(sq(sq(sq((C2 * Src0 + C1) * Src0 + C0))))

    def _ref(in0, in1, s0, s1, imm2):
        q = (imm2 * in0 + s1) * in0 + s0
        return q ** 16

    spec = Spec(body=body, reference=_ref)
    opcode = dops._CUSTOM_DVE_ROW_BASE + len(dops.OPS)
    shas = {}
    for ver in ("v3", "v4"):
        try:
            compiled = DveOpSpec(name=name, opcode=opcode,
                                 uops=lower(spec, ver=ver),
                                 rd1_en=_has_src1(spec))
            shas[ver] = compiled.sha(ver)
        except Exception:
            pass
    op = dops.DveOp(name, spec, subdim=False, uops_sha=shas)
    dops.OPS.append(op)
    dops.CUSTOM_DVE_SPECS[name] = spec
    dops._SUB_OPCODE_FOR_NAME[name] = opcode
    return op


def _build_program():
    import concourse.bacc as bacc
    import concourse.tile as tile
    from concourse import mybir

    dtf = mybir.dt.float32
    dtr = mybir.dt.float32r
    dtb = mybir.dt.bfloat16
    dt8 = mybir.dt.float8e4
    EXP = mybir.ActivationFunctionType.Exp
    MULT = mybir.AluOpType.mult
    DR = mybir.MatmulPerfMode.DoubleRow

    exp16 = _register_exp16()
    ec0, ec1, ec2 = _fit_exp16_constants(EXP_SCALE)
    expctr = {'i': 0}

    nc = bacc.Bacc('TRN2', target_bir_lowering=False, debug=False)
    xt8_d = nc.dram_tensor("xt8", [1024, 2048], dt8, kind="ExternalInput").ap()
    xt_d = nc.dram_tensor("xt", [1024, 2048], dtb, kind="ExternalInput").ap()
    wqk_d = nc.dram_tensor("wqk", [1024, 1024], dt8, kind="ExternalInput").ap()
    wv_d = nc.dram_tensor("wv", [1024, 512], dtb, kind="ExternalInput").ap()
    wout_d = nc.dram_tensor("wout", [512, 1024], dtb, kind="ExternalInput").ap()
    mask_d = nc.dram_tensor("mask", [4, 128, 512], dtb, kind="ExternalInput").ap()
    y_d = nc.dram_tensor("y", [2048, 1024], dtf, kind="ExternalOutput").ap()

    with tile.TileContext(nc) as tc:
        with tc.tile_pool(name="ps_s", bufs=2, space="PSUM") as ps_s, \
             tc.tile_pool(name="ps", bufs=2, space="PSUM") as ps, \
             tc.tile_pool(name="ps_ctx", bufs=2, space="PSUM") as ps_ctx, \
             tc.tile_pool(name="const", bufs=1) as const, \
             tc.tile_pool(name="xt_p", bufs=16) as xt_p, \
             tc.tile_pool(name="xt8_p", bufs=8) as xt8_p, \
             tc.tile_pool(name="qt_p", bufs=8) as qt_p, \
             tc.tile_pool(name="exp_p", bufs=8) as exp_p, \
             tc.tile_pool(name="cn_p", bufs=16) as cn_p, \
             tc.tile_pool(name="row_p", bufs=4) as row_p, \
             tc.tile_pool(name="rcp_p", bufs=2) as rcp_p, \
             tc.tile_pool(name="y_p", bufs=2) as y_p:

            # ---- constants / weights ----
            # wqk_sb[p, kp, i, oc, c]: fp8 DR layout; rows 256*kp+128*i+p of
            # Wqk, cols oc*128+c. oc 0-3 Q, 4-7 K.
            wqk_sb = const.tile([128, 4, 2, 8, 128], dt8)
            wv_sb = const.tile([128, 8, 512], dtb)       # [p, kc, n]
            wout_sb = const.tile([128, 4, 2, 512], dtb)  # [p, hp, oc, c]
            masks = const.tile([128, 4, 512], dtb)
            ones_f32 = const.tile([1, 128], dtf)
            ones_t = const.tile([1, 128], dtr)
            kt_store = const.tile([128, 4, 4, 512], dt8)  # [p, hp, tt, t]
            v_all = const.tile([128, 16, 8, 65], dtb)     # [p, kt, h, d|1]

            nc.vector.memset(v_all[:, :, :, 64:65], 1.0)
            nc.vector.memset(ones_f32[:], 1.0)
            nc.vector.tensor_copy(ones_t[:], ones_f32[:])

            qts = {}   # tt -> [4 qt tiles (fp8)]
            xts = {}   # tt -> [8 bf16 xt tiles]
            x8s = {}   # tt -> [4 fp8 xt DR tiles]
            cns = {}   # qt -> [4 cn tiles]

            def emit_xt_dma(tt):
                ts = []
                for kc in range(8):
                    t_ = xt_p.tile([128, 512], dtb, tag="xt")
                    nc.sync.dma_start(
                        t_[:], xt_d[kc * 128:(kc + 1) * 128,
                                    tt * 512:(tt + 1) * 512])
                    ts.append(t_)
                xts[tt] = ts
                t8s = []
                for kp in range(4):
                    t_ = xt8_p.tile([128, 2, 512], dt8, tag="xt8")
                    nc.scalar.dma_start(
                        t_[:], xt8_d[kp * 256:(kp + 1) * 256,
                                     tt * 512:(tt + 1) * 512]
                        .rearrange("(i p) t -> p i t", p=128))
                    t8s.append(t_)
                x8s[tt] = t8s

            # DMA order: Q-half wqk chunks (sync) + xt0 feed the first Q
            # matmuls ASAP; K-half wqk rides the gpsimd queue.
            xts[0] = []
            x8s[0] = []
            for kp in range(4):
                nc.sync.dma_start(
                    wqk_sb[:, kp, :, 0:4, :],
                    wqk_d[kp * 256:(kp + 1) * 256, 0:512]
                    .rearrange("(i p) (oc c) -> p i oc c", p=128, c=128))
                t_ = xt8_p.tile([128, 2, 512], dt8, tag="xt8", name="xt80")
                nc.scalar.dma_start(
                    t_[:], xt8_d[kp * 256:(kp + 1) * 256, 0:512]
                    .rearrange("(i p) t -> p i t", p=128))
                x8s[0].append(t_)
                nc.gpsimd.dma_start(
                    wqk_sb[:, kp, :, 4:8, :],
                    wqk_d[kp * 256:(kp + 1) * 256, 512:1024]
                    .rearrange("(i p) (oc c) -> p i oc c", p=128, c=128))
            for kc in range(8):
                t_ = xt_p.tile([128, 512], dtb, tag="xt", name="xt0")
                nc.scalar.dma_start(
                    t_[:], xt_d[kc * 128:(kc + 1) * 128, 0:512])
                xts[0].append(t_)
            nc.sync.dma_start(wv_sb[:],
                              wv_d.rearrange("(kc p) n -> p kc n", p=128))
            nc.sync.dma_start(masks[:], mask_d.rearrange("n p f -> p n f"))
            emit_xt_dma(1)
            nc.sync.dma_start(wout_sb[:],
                              wout_d.rearrange("(hp p) (oc c) -> p hp oc c",
                                               p=128, c=512))

            def proj_closures(tt):
                cl = []
                qts[tt] = [None] * 4
                state = {}

                def q_mm(j, kp):
                    if kp == 0:
                        state[('q', j)] = ps.tile([128, 512], dtf, tag="ps", name="psq")
                    psq = state[('q', j)]
                    nc.tensor.matmul(psq[:], wqk_sb[:, kp, :, j, :],
                                     x8s[tt][kp][:], start=(kp == 0),
                                     stop=(kp == 3),
                                     perf_mode=DR).annotate('mm_q')
                    if kp == 3:
                        qt_t = qt_p.tile([128, 512], dt8, tag="qt")
                        nc.vector.tensor_copy(qt_t[:], psq[:])
                        qts[tt][j] = qt_t

                def k_mm(j, kp):
                    if kp == 0:
                        state[('k', j)] = ps.tile([128, 512], dtf, tag="ps", name="psk")
                    psk = state[('k', j)]
                    nc.tensor.matmul(psk[:], wqk_sb[:, kp, :, 4 + j, :],
                                     x8s[tt][kp][:], start=(kp == 0),
                                     stop=(kp == 3),
                                     perf_mode=DR).annotate('mm_k')
                    if kp == 3:
                        nc.vector.tensor_copy(kt_store[:, j, tt, :], psk[:])

                def v_mm(sub, kc):
                    if kc == 0:
                        state[('v', sub)] = ps.tile([128, 512], dtf, tag="ps", name="psv")
                    psv = state[('v', sub)]
                    nc.tensor.matmul(psv[:],
                                     xts[tt][kc][:, sub * 128:(sub + 1) * 128],
                                     wv_sb[:, kc, :], start=(kc == 0),
                                     stop=(kc == 7)).annotate('mm_v')
                    if kc == 7:
                        vt = tt * 4 + sub
                        nc.vector.tensor_copy(
                            v_all[:, vt, :, 0:64],
                            psv[:].rearrange("p (h d) -> p h d", h=8))

                for j in range(4):
                    for kp in range(4):
                        cl.append(lambda j=j, kp=kp: q_mm(j, kp))
                for j in range(4):
                    for kp in range(4):
                        cl.append(lambda j=j, kp=kp: k_mm(j, kp))
                for sub in range(4):
                    for kc in range(8):
                        cl.append(lambda s=sub, kc=kc: v_mm(s, kc))
                return cl

            def y_closures(qt):
                cl = []
                state = {}

                def y_mm(mi, oc, hp):
                    if hp == 0:
                        state[(mi, oc)] = ps.tile([128, 512], dtf, tag="ps", name="psy")
                    psy = state[(mi, oc)]
                    nc.tensor.matmul(psy[:],
                                     cns[qt][hp][:, mi * 128:(mi + 1) * 128],
                                     wout_sb[:, hp, oc, :],
                                     start=(hp == 0), stop=(hp == 3)).annotate('mm_y')
                    if hp == 3:
                        y_sb = y_p.tile([128, 512], dtf, tag="y")
                        nc.vector.tensor_copy(y_sb[:], psy[:])
                        nc.sync.dma_start(
                            y_d[qt * 512 + mi * 128: qt * 512 + (mi + 1) * 128,
                                oc * 512:(oc + 1) * 512],
                            y_sb[:])

                for mi in range(4):
                    for oc in range(2):
                        for hp in range(4):
                            cl.append(lambda m=mi, o=oc, h=hp: y_mm(m, o, h))
                return cl

            def unit2(qt, hp, cn_t, pull):
                """Both hb heads of pair hp, interleaved so mm_s for hb=0
                (rows 0-63) and hb=1 (rows 64-127) are pc-adjacent and run
                concurrently on disjoint PE row groups."""
                n_kt = 4 * qt + 4
                np_ = n_kt // 2
                ctx = [ps_ctx.tile([65, 512], dtf, tag="ctx", name=f"ctx{hb}")
                       for hb in (0, 1)]
                pending = []

                def off(kt):
                    di = kt - 4 * qt
                    return 128 * di if di > 0 else 0

                def emit_ctx(j, exs):
                    for hb in (0, 1):
                        for t in (0, 1):
                            kt = 2 * j + t
                            o = off(kt)
                            nc.tensor.matmul(ctx[hb][:, o:512],
                                             v_all[:, kt, 2 * hp + hb, :],
                                             exs[hb][:, t, o:512],
                                             start=(kt == 0),
                                             stop=(kt == n_kt - 1)).annotate('mm_ctx')

                for j in range(np_):
                    sps = [ps_s.tile([128, 2, 512], dtf, tag="s", name=f"sp{hb}")
                           for hb in (0, 1)]
                    for t in (0, 1):
                        kt = 2 * j + t
                        ktt, kj = kt // 4, kt % 4
                        o = off(kt)
                        for hb in (0, 1):
                            pb = hb * 64
                            nc.tensor.matmul(
                                sps[hb][:, t, o:512],
                                kt_store[pb:pb + 64, hp, ktt,
                                         kj * 128:(kj + 1) * 128],
                                qts[qt][hp][pb:pb + 64, o:512],
                                start=True, stop=True).annotate('mm_s')
                    pull(4)
                    exs = []
                    for hb in (0, 1):
                        ex = exp_p.tile([128, 2, 512], dtb, tag="exp")
                        expctr['i'] += 1
                        on_dve = DVE_EXP_EVERY and (expctr['i'] % DVE_EXP_EVERY) == 0

                        def _exp(dst, src):
                            if on_dve:
                                nc.vector._custom_dve(exp16, out=dst, in0=src,
                                                      s0=ec0, s1=ec1, imm2=ec2)
                            else:
                                nc.scalar.activation(dst, src, EXP,
                                                     scale=EXP_SCALE)
                        if off(2 * j) >= 256:
                            for t in (0, 1):
                                o = off(2 * j + t)
                                _exp(ex[:, t, o:512], sps[hb][:, t, o:512])
                        else:
                            _exp(ex[:], sps[hb][:])
                        for t in (0, 1):
                            kt = 2 * j + t
                            di = kt - 4 * qt
                            if 0 <= di < 4:
                                o, w = 128 * di, 128 * (di + 1)
                                nc.vector.tensor_tensor(ex[:, t, o:w],
                                                        ex[:, t, o:w],
                                                        masks[:, di, o:w], MULT)
                        exs.append(ex)
                    pending.append((j, exs))
                    if len(pending) > 2:
                        emit_ctx(*pending.pop(0))
                        pull(2)
                while pending:
                    emit_ctx(*pending.pop(0))
                    pull(2)
                # normalize both heads' halves of cn
                for hb in (0, 1):
                    pb = hb * 64
                    if qt == 3 and hp >= 2:
                        # final units feed the epilogue now: broadcast on PE
                        row = row_p.tile([1, 512], dtr, tag="row")
                        nc.vector.tensor_copy(row[:], ctx[hb][64:65, :])
                        bc = ps.tile([128, 512], dtf, tag="ps", name="bc")
                        nc.tensor.matmul(bc[:], ones_t[:], row[:],
                                         start=True, stop=True).annotate('mm_bc')
                        rcp = rcp_p.tile([64, 512], dtf, tag="rcp", name="rcp")
                        nc.vector.reciprocal_approx_fast(rcp[:], bc[0:64, :])
                        nc.vector.tensor_tensor(cn_t[pb:pb + 64, :],
                                                ctx[hb][0:64, :], rcp[:], MULT)
                        continue
                    row = row_p.tile([1, 512], dtf, tag="row")
                    nc.vector.tensor_copy(row[:], ctx[hb][64:65, :])
                    rrow = row_p.tile([1, 512], dtf, tag="row", name="rrow")
                    nc.vector.reciprocal_approx_fast(rrow[:], row[:])
                    pull(1)
                    rcp = rcp_p.tile([64, 512], dtf, tag="rcp")
                    nc.gpsimd.partition_broadcast(rcp[:], rrow[:])
                    nc.vector.tensor_tensor(cn_t[pb:pb + 64, :],
                                            ctx[hb][0:64, :], rcp[:], MULT)

            def make_puller(items, total_slots):
                st = {'i': 0, 'slot': 0}
                n = len(items)

                def pull(k):
                    st['slot'] += k
                    if total_slots > 0:
                        target = min(n, (n * st['slot'] + total_slots - 1)
                                     // total_slots)
                    else:
                        target = n
                    # burst to a multiple of 8: longer same-type matmul runs
                    # keep the PE weight double-buffer streaming
                    if st['i'] < target:
                        target = min(n, ((target + 7) // 8) * 8)
                    while st['i'] < target:
                        items[st['i']]()
                        st['i'] += 1

                def drain():
                    while st['i'] < n:
                        items[st['i']]()
                        st['i'] += 1
                return pull, drain

            # ---- prologue: tt=0 projections ----
            warm = row_p.tile([1, 1], dtf, tag="row", name="warm")
            nc.scalar.activation(warm[:], v_all[0:1, 0, 0, 64:65], EXP)
            for c in proj_closures(0):
                c()

            # ---- main loop ----
            for qt in range(4):
                Bq = []
                if qt < 2:
                    emit_xt_dma(qt + 2)
                if qt < 3:
                    Bq += proj_closures(qt + 1)
                if qt == 3:
                    Bq += (y_closures(0) + y_closures(1) + y_closures(2)
                           + y_closures(3))
                np_ = (4 * qt + 4) // 2
                total_slots = 4 * (8 * np_ + 2)
                pull, drain = make_puller(Bq, total_slots)
                # pre-register all four cn tiles so late-paced y(3)
                # closures can reference cns[3][hp] before unit2(hp) runs;
                # the tile framework orders mm_y after the cn writes.
                cns[qt] = [cn_p.tile([128, 512], dtb, tag="cn", name=f"cn{hp}")
                           for hp in range(4)]
                for hp in range(4):
                    unit2(qt, hp, cns[qt][hp], pull)
                drain()
    nc.compile()
    return nc


def _host_shards(x, Wqkv, bqkv, Wout):
    import ml_dtypes
    mask = np.zeros((4, 128, 512), np.float32)
    qq = np.arange(512)[None, :]
    kk = np.arange(128)[:, None]
    for di in range(4):
        mask[di] = (kk + di * 128 <= qq)
    mask = mask.astype(ml_dtypes.bfloat16)

    assert not np.any(bqkv), "kernel assumes zero qkv bias"

    in_maps = []
    for c in range(NCORES):
        b, hg = c // 2, c % 2
        s = hg * 512
        xt_f = np.ascontiguousarray(x[b].T)
        xt8 = xt_f.astype(ml_dtypes.float8_e4m3)
        xt = xt_f.astype(ml_dtypes.bfloat16)
        wqk = np.ascontiguousarray(
            np.concatenate([Wqkv[:, s:s + 512] * WSCALE,
                            Wqkv[:, 1024 + s:1024 + s + 512] * WSCALE],
                           axis=1)).astype(ml_dtypes.float8_e4m3)
        wv = np.ascontiguousarray(Wqkv[:, 2048 + s:2048 + s + 512]).astype(ml_dtypes.bfloat16)
        wout = np.ascontiguousarray(Wout[s:s + 512, :]).astype(ml_dtypes.bfloat16)
        in_maps.append({"xt8": xt8, "xt": xt, "wqk": wqk, "wv": wv,
                        "wout": wout, "mask": mask})
    return in_maps


_CACHED = {}


def kernel(x, Wqkv, bqkv, Wout, bout):
    from concourse.bass_utils import run_bass_kernel_spmd

    x = np.asarray(x, dtype=np.float32)
    Wqkv = np.asarray(Wqkv, dtype=np.float32)
    bqkv = np.asarray(bqkv, dtype=np.float32)
    Wout = np.asarray(Wout, dtype=np.float32)
    bout = np.asarray(bout, dtype=np.float32)
    assert x.shape == (B, T, C), x.shape

    if 'nc' not in _CACHED:
        _CACHED['nc'] = _build_program()
    nc = _CACHED['nc']

    in_maps = _host_shards(x, Wqkv, bqkv, Wout)
    res = run_bass_kernel_spmd(nc, in_maps, core_ids=list(range(NCORES)))

    y = np.empty((B, T, C), np.float32)
    for b in range(B):
        y[b] = res.results[2 * b]["y"] + res.results[2 * b + 1]["y"] + bout
    return y
